# revision 1
# baseline (speedup 1.0000x reference)
"""Trainium2 Bass kernel for nn_Decoder_22703197127089 (moe_routing).

Key insight: the module's output depends only on each sample's LAST token
(h[:, -1, :] is taken after the MoE block), so the MoE block and all
attention rows except the last are dead code.  What remains per sample:
  conv1d patch embed (all 1023 tokens)  -> LN1 -> k,v projections (all
  tokens) + q for the last token -> one attention row -> out-proj ->
  MoE for 1 token -> LN2 -> final linear (96).

Sharding: data-parallel over batch B=32 across 8 cores (4 samples/core).
No collectives; host gathers the (4, 96) per-core outputs.

Layouts on device (per core):
  - X2 tile (128, L): partitions 0-63 = X[c, l], 64-127 = X[c, l+1]
    so that conv contraction chunks of 128 = (p in {2k, 2k+1}) x (c in
    0..63) are strided APs (offset 2k, stride 12 over patches).
  - conv output h0T is feature-major (64, N) per sample; two samples of
    a pair live stacked in one (128, N) tile (col-tiled matmul pair).
  - LN1 is folded into the projections:
      k = LN(h0) @ kw.T  =>  scores use k0 = kw @ h0T plus per-token
      (mu, rstd) corrections; v = LN(h0) @ vw.T folded into the
      attention weighted sum.  Per-token stats come from PE matmuls
      against a [ones;0]/[0;ones] selector (cross-partition reduce).
"""

import numpy as np

import concourse.bass as bass
import concourse.mybir as mybir
import concourse.tile as tile
from concourse import bacc
from concourse.bass_utils import run_bass_kernel_spmd

F32 = mybir.dt.float32
AF = mybir.ActivationFunctionType
OP = mybir.AluOpType

B, C, L = 32, 64, 12288
D = 64
E = 8
TOPK = 4
P, S = 24, 12
PRED = 96
N = (L - P) // S + 1  # 1023
NCORES = 8
SPC = B // NCORES     # 4 samples per core
NPAIR = SPC // 2      # 2
NCH = (C * P) // 128  # 12 contraction chunks of K=128 (p-pair, c)
NT = 1024             # padded token dim (col 1023 zeroed)
NJ = 8                # 128-token chunks
EPS = 1e-5
MCS = (512, 511)      # conv/k0 m-chunk sizes
DT_X = mybir.dt.bfloat16  # X/conv-weight compute dtype (F32 for exact)


def _pos_encoding_np(n, d):
    # match reference._pos_encoding in float32
    pos = np.arange(n, dtype=np.float32)[:, None]
    div = np.exp(np.arange(0, d, 2, dtype=np.float32)
                 * (np.float32(-np.log(np.float32(10000.0))) / np.float32(d)))
    pe = np.zeros((n, d), np.float32)
    pe[:, 0::2] = np.sin(pos * div)
    pe[:, 1::2] = np.cos(pos * div)
    return pe


def build_nc(debug_taps=False):
    nc = bacc.Bacc("TRN2", target_bir_lowering=False, debug=False,
                   num_devices=NCORES)

    inp = {}
    def di(name, shape, dtype=F32):
        inp[name] = nc.dram_tensor(name, list(shape), dtype,
                                   kind="ExternalInput")
        return inp[name]

    Xs = di("Xs", (SPC, C, L), DT_X)
    Wc = di("Wc", (C * P, D), DT_X)       # (p,c)-ordered conv weight
    PEBT2 = di("PEBT2", (128, N))          # [pebT; pebT]
    QwT2 = di("QwT2", (128, D))
    Kw2 = di("Kw2", (128, D))
    VwT2 = di("VwT2", (128, D))
    OwT = di("OwT", (D, D))                # ow.T
    SqCol2 = di("SqCol2", (128, 1))        # [qw.sum(1); qw.sum(1)]
    SkCol2 = di("SkCol2", (128, 1))
    SvCol = di("SvCol", (D, 1))
    SelAB = di("SelAB", (128, 2))          # [[1;0],[0;1]] selector
    OnesRow = di("OnesRow", (1, 128))
    Ones128 = di("Ones128", (128, 1))
    OneHot = di("OneHot", (128, 1))        # 1.0 at partition 126
    LastMask = di("LastMask", (128, 1))    # ones, 0.0 at partition 127
    RwT = di("RwT", (D, E))
    WexpE = di("WexpE", (D + 1, E * D))    # experts + bias row
    MowT = di("MowT", (D, D))
    OutWT = di("OutWT", (D, PRED))
    Id4 = di("Id4", (SPC, SPC))

    Yout = nc.dram_tensor("Yout", [SPC, PRED], F32, kind="ExternalOutput")
    taps = {}

    with tile.TileContext(nc) as tc:
        with (
            tc.tile_pool(name="const", bufs=1) as pc,
            tc.tile_pool(name="xp", bufs=4) as xp,
            tc.tile_pool(name="hp", bufs=2) as hp,
            tc.tile_pool(name="sqp", bufs=2) as sqp,
            tc.tile_pool(name="vp", bufs=2) as vp,
            tc.tile_pool(name="sm", bufs=2) as sm,
            tc.tile_pool(name="ps", bufs=1, space="PSUM") as ps,
        ):
            # ---- constants to SBUF ----
            wsb = pc.tile([128, NCH * D], DT_X, tag="wsb")
            nc.gpsimd.dma_start(
                wsb[:].rearrange("p (k d) -> p k d", k=NCH),
                Wc.ap().rearrange("(k p) d -> p k d", p=128))
            pebt = pc.tile([128, N], F32, tag="pebt")
            nc.gpsimd.dma_start(pebt[:], PEBT2.ap())
            qwt = pc.tile([128, D], F32, tag="qwt")
            nc.gpsimd.dma_start(qwt[:], QwT2.ap())
            kw2 = pc.tile([128, D], F32, tag="kw2")
            nc.gpsimd.dma_start(kw2[:], Kw2.ap())
            vwt = pc.tile([128, D], F32, tag="vwt")
            nc.gpsimd.dma_start(vwt[:], VwT2.ap())
            owt = pc.tile([D, D], F32, tag="owt")
            nc.gpsimd.dma_start(owt[:], OwT.ap())
            sqcol = pc.tile([128, 1], F32, tag="sqcol")
            nc.gpsimd.dma_start(sqcol[:], SqCol2.ap())
            skcol = pc.tile([128, 1], F32, tag="skcol")
            nc.gpsimd.dma_start(skcol[:], SkCol2.ap())
            svcol = pc.tile([D, 1], F32, tag="svcol")
            nc.gpsimd.dma_start(svcol[:], SvCol.ap())
            selab = pc.tile([128, 2], F32, tag="selab")
            nc.gpsimd.dma_start(selab[:], SelAB.ap())
            onesr = pc.tile([1, 128], F32, tag="onesr")
            nc.gpsimd.dma_start(onesr[:], OnesRow.ap())
            ones128 = pc.tile([128, 1], F32, tag="ones128")
            nc.gpsimd.dma_start(ones128[:], Ones128.ap())
            onehot = pc.tile([128, 1], F32, tag="onehot")
            nc.gpsimd.dma_start(onehot[:], OneHot.ap())
            lastm = pc.tile([128, 1], F32, tag="lastm")
            nc.gpsimd.dma_start(lastm[:], LastMask.ap())
            rwt = pc.tile([D, E], F32, tag="rwt")
            nc.gpsimd.dma_start(rwt[:], RwT.ap())
            wexp = pc.tile([D + 1, E * D], F32, tag="wexp")
            nc.gpsimd.dma_start(wexp[:], WexpE.ap())
            mowt = pc.tile([D, D], F32, tag="mowt")
            nc.gpsimd.dma_start(mowt[:], MowT.ap())
            outwt = pc.tile([D, PRED], F32, tag="outwt")
            nc.gpsimd.dma_start(outwt[:], OutWT.ap())
            id4 = pc.tile([SPC, SPC], F32, tag="id4")
            nc.gpsimd.dma_start(id4[:], Id4.ap())

            # attention outputs of all samples, + ones row for expert bias
            ha = pc.tile([D + 1, SPC], F32, tag="ha")
            nc.vector.memset(ha[D:D + 1, :], 1.0)
            epsb = pc.tile([128, 1], F32, tag="epsb")
            nc.vector.memset(epsb[:], EPS)

            for pair in range(NPAIR):
                # ---- X load: per-sample tile, partitions 64-127 hold
                # X shifted left by one so K=128 chunks cover (p, p+1) pairs
                XSPLIT = 6656
                x2 = []
                for s01 in range(2):
                    t = xp.tile([128, L], DT_X, tag="x2", name="x2t")
                    nc.sync.dma_start(t[0:C, 0:XSPLIT],
                                      Xs.ap()[2 * pair + s01][:, 0:XSPLIT])
                    nc.sync.dma_start(t[0:C, XSPLIT:L],
                                      Xs.ap()[2 * pair + s01][:, XSPLIT:L])
                    nc.sync.dma_start(t[C:128, 0:XSPLIT - 1],
                                      t[0:C, 1:XSPLIT])
                    nc.sync.dma_start(t[C:128, XSPLIT - 1:L - 1],
                                      t[0:C, XSPLIT:L])
                    x2.append(t)

                # ---- conv -> h0T pair (feature-major, A rows 0-63 / B 64-127)
                h0 = hp.tile([128, NT], F32, tag="h0")
                nc.vector.memset(h0[:, N:NT], 0.0)
                n0 = 0
                xv = [t[:].rearrange("p (n s) -> p n s", s=S) for t in x2]
                for mc, nn in enumerate(MCS):
                    cps = ps.tile([128, 512], F32, tag="convA", name="cps", bufs=2)
                    for k in range(NCH):
                        q, r = divmod(2 * k, S)
                        for s01 in range(2):
                            o = 64 * s01
                            nc.tensor.matmul(
                                cps[o:o + 64, 0:nn],
                                lhsT=wsb[:, D * k:D * k + D],
                                rhs=xv[s01][:, n0 + q:n0 + q + nn, r],
                                start=(k == 0), stop=(k == NCH - 1))
                    nc.vector.tensor_add(h0[:, n0:n0 + nn], cps[:, 0:nn],
                                         pebt[:, n0:n0 + nn])
                    n0 += nn

                # ---- LN1 stats: per-token colsum & sum-of-squares ----
                sq = sqp.tile([128, NT], F32, tag="sq")
                nc.scalar.activation(sq[:], h0[:], AF.Square)
                stp = ps.tile([128, 32], F32, tag="stats", bufs=2)
                for j in range(NJ):
                    nc.tensor.matmul(stp[:, 2 * j:2 * j + 2],
                                     lhsT=h0[:, 128 * j:128 * j + 128],
                                     rhs=selab[:], start=True, stop=True)
                    nc.tensor.matmul(stp[:, 16 + 2 * j:16 + 2 * j + 2],
                                     lhsT=sq[:, 128 * j:128 * j + 128],
                                     rhs=selab[:], start=True, stop=True)
                mean = sm.tile([128, 16], F32, tag="mean")
                nc.vector.tensor_scalar_mul(mean[:], stp[:, 0:16], 1.0 / D)
                ex2 = sm.tile([128, 16], F32, tag="ex2")
                nc.vector.tensor_scalar_mul(ex2[:], stp[:, 16:32], 1.0 / D)
                var = sm.tile([128, 16], F32, tag="var")
                nc.vector.tensor_mul(var[:], mean[:], mean[:])
                nc.vector.tensor_sub(var[:], ex2[:], var[:])
                std = sm.tile([128, 16], F32, tag="std")
                nc.scalar.activation(std[:], var[:], AF.Sqrt, bias=epsb[:])
                rstd = sm.tile([128, 16], F32, tag="rstd")
                nc.vector.reciprocal(rstd[:], std[:])
                r8 = sm.tile([128, 16], F32, tag="r8")
                nc.vector.tensor_scalar_mul(r8[:], rstd[:], 0.125)

                # ---- attention row, both samples of the pair packed ----
                # misc psum column map:
                #  0 q0(A rows 0:64 / B rows 64:128); 2-3 cb pair; 4-19 scores
                #  (col 4+2j+t); 20-23 extracts muA,rA,muB,rB; 24 mu bcast,
                #  25 r bcast (per-sample rows); 26-27 c1 A/B; 28 z2(2x1);
                #  29-30 zrow(1x2); 32-33 rzb(128x2); 34 g2; 35-36 grow;
                #  37-38 gb(64x2); 39-40 att A/B; 41-42 proj A/B
                misc = ps.tile([128, 44], F32, tag="misc", bufs=2)
                qe = sm.tile([128, 1], F32, tag="qe")
                for s01 in range(2):
                    o = 64 * s01
                    nc.tensor.matmul(misc[o:o + 64, 0:1], lhsT=qwt[o:o + 64, :],
                                     rhs=h0[o:o + 64, N - 1:N],
                                     start=True, stop=True)
                    nc.tensor.matmul(misc[0:1, 20 + 2 * s01:21 + 2 * s01],
                                     lhsT=mean[:, 14 + s01:15 + s01],
                                     rhs=onehot[:], start=True, stop=True)
                    nc.tensor.matmul(misc[0:1, 21 + 2 * s01:22 + 2 * s01],
                                     lhsT=rstd[:, 14 + s01:15 + s01],
                                     rhs=onehot[:], start=True, stop=True)
                ex4 = sm.tile([1, 4], F32, tag="ex4")
                nc.vector.tensor_copy(ex4[:], misc[0:1, 20:24])
                for s01 in range(2):
                    o = 64 * s01
                    # broadcast mu_last, r_last to this sample's 64 partitions
                    nc.tensor.matmul(misc[o:o + 64, 24:25],
                                     lhsT=onesr[0:1, 0:64],
                                     rhs=ex4[0:1, 2 * s01:2 * s01 + 1],
                                     start=True, stop=True)
                    nc.tensor.matmul(misc[o:o + 64, 25:26],
                                     lhsT=onesr[0:1, 0:64],
                                     rhs=ex4[0:1, 2 * s01 + 1:2 * s01 + 2],
                                     start=True, stop=True)
                    # q_eff = r_last * (q0 - mu_last * Sq)
                    nc.vector.tensor_mul(qe[o:o + 64, :], sqcol[o:o + 64, :],
                                         misc[o:o + 64, 24:25])
                    nc.vector.tensor_sub(qe[o:o + 64, :], misc[o:o + 64, 0:1],
                                         qe[o:o + 64, :])
                    nc.vector.tensor_mul(qe[o:o + 64, :], qe[o:o + 64, :],
                                         misc[o:o + 64, 25:26])
                    nc.tensor.matmul(misc[0:1, 26 + s01:27 + s01],
                                     lhsT=qe[o:o + 64, :],
                                     rhs=skcol[o:o + 64, :],
                                     start=True, stop=True)
                    # qk = kw.T @ q_eff: scores[m] = qk . h0T[:, m]
                    nc.tensor.matmul(misc[o:o + 64, 1:2],
                                     lhsT=kw2[o:o + 64, :],
                                     rhs=qe[o:o + 64, :],
                                     start=True, stop=True)
                qks = sm.tile([128, 1], F32, tag="qks")
                nc.vector.tensor_copy(qks[:], misc[:, 1:2])
                c1r = sm.tile([1, 2], F32, tag="c1r")
                nc.vector.tensor_copy(c1r[:], misc[0:1, 26:28])
                nc.tensor.matmul(misc[:, 2:4], lhsT=onesr[:], rhs=c1r[:],
                                 start=True, stop=True)
                # scores for both samples, interleaved like the stats tiles
                for j in range(NJ):
                    for s01 in range(2):
                        o = 64 * s01
                        nc.tensor.matmul(
                            misc[:, 4 + 2 * j + s01:5 + 2 * j + s01],
                            lhsT=h0[o:o + 64, 128 * j:128 * j + 128],
                            rhs=qks[o:o + 64, :], start=True, stop=True)
                mv3 = mean[:].rearrange("p (j t) -> p j t", t=2)
                tmp = sm.tile([128, 16], F32, tag="tmp")
                nc.vector.tensor_tensor(
                    tmp[:].rearrange("p (j t) -> p j t", t=2), mv3,
                    misc[:, 2:4][:, None].to_broadcast([128, NJ, 2]),
                    op=OP.mult)
                sc = sm.tile([128, 16], F32, tag="sc")
                nc.vector.tensor_sub(sc[:], misc[:, 4:20], tmp[:])
                nc.vector.tensor_mul(sc[:], sc[:], r8[:])
                exps = sm.tile([128, 16], F32, tag="exps")
                nc.scalar.activation(exps[:], sc[:], AF.Exp)
                nc.vector.tensor_scalar(exps[:, 14:16], exps[:, 14:16],
                                        lastm[:], None, op0=OP.mult)
                # Z per sample: reduce over chunks, then over partitions
                zs = sm.tile([128, 2], F32, tag="zs")
                nc.vector.tensor_reduce(
                    zs[:], exps[:].rearrange("p (j t) -> p t j", t=2),
                    mybir.AxisListType.X, OP.add)
                nc.tensor.matmul(misc[0:2, 28:29], lhsT=zs[:], rhs=ones128[:],
                                 start=True, stop=True)
                z2s = sm.tile([2, 1], F32, tag="z2s")
                nc.vector.tensor_copy(z2s[:], misc[0:2, 28:29])
                nc.tensor.matmul(misc[0:1, 29:31], lhsT=z2s[:],
                                 rhs=id4[0:2, 0:2], start=True, stop=True)
                rzr = sm.tile([1, 2], F32, tag="rzr")
                nc.vector.reciprocal(rzr[:], misc[0:1, 29:31])
                nc.tensor.matmul(misc[:, 32:34], lhsT=onesr[:], rhs=rzr[:],
                                 start=True, stop=True)
                # wr = exps/Z * rstd ; g = sum(wr * mu) per sample
                wr = sm.tile([128, 16], F32, tag="wr")
                nc.vector.tensor_tensor(
                    wr[:].rearrange("p (j t) -> p j t", t=2),
                    exps[:].rearrange("p (j t) -> p j t", t=2),
                    misc[:, 32:34][:, None].to_broadcast([128, NJ, 2]),
                    op=OP.mult)
                nc.vector.tensor_mul(wr[:], wr[:], rstd[:])
                gt = sm.tile([128, 16], F32, tag="gt")
                nc.vector.tensor_mul(gt[:], wr[:], mean[:])
                gs = sm.tile([128, 2], F32, tag="gs")
                nc.vector.tensor_reduce(
                    gs[:], gt[:].rearrange("p (j t) -> p t j", t=2),
                    mybir.AxisListType.X, OP.add)
                nc.tensor.matmul(misc[0:2, 34:35], lhsT=gs[:], rhs=ones128[:],
                                 start=True, stop=True)
                g2s = sm.tile([2, 1], F32, tag="g2s")
                nc.vector.tensor_copy(g2s[:], misc[0:2, 34:35])
                nc.tensor.matmul(misc[0:1, 35:37], lhsT=g2s[:],
                                 rhs=id4[0:2, 0:2], start=True, stop=True)
                grow = sm.tile([1, 2], F32, tag="grow")
                nc.vector.tensor_copy(grow[:], misc[0:1, 35:37])
                nc.tensor.matmul(misc[0:64, 37:39], lhsT=onesr[0:1, 0:64],
                                 rhs=grow[:], start=True, stop=True)
                # v0 + attention accumulate + out-proj, per sample
                for s01 in range(2):
                    s = 2 * pair + s01
                    o = 64 * s01
                    vps = ps.tile([128, 512], F32, tag="v0", bufs=2)
                    for j in range(NJ):
                        nc.tensor.matmul(vps[:, 64 * j:64 * j + 64],
                                         lhsT=h0[o:o + 64, 128 * j:128 * j + 128],
                                         rhs=vwt[o:o + 64, :],
                                         start=True, stop=True)
                    v0 = vp.tile([128, 512], F32, tag="v0sb")
                    nc.scalar.copy(v0[:], vps[:])
                    for j in range(NJ):
                        nc.tensor.matmul(
                            misc[0:64, 39 + s01:40 + s01],
                            lhsT=v0[:, 64 * j:64 * j + 64],
                            rhs=wr[:, 2 * j + s01:2 * j + s01 + 1],
                            start=(j == 0), stop=(j == NJ - 1))
                    oc = sm.tile([64, 1], F32, tag="oc")
                    nc.vector.tensor_mul(oc[:], svcol[:],
                                         misc[0:64, 37 + s01:38 + s01])
                    nc.vector.tensor_sub(oc[:], misc[0:64, 39 + s01:40 + s01],
                                         oc[:])
                    nc.tensor.matmul(misc[0:64, 41 + s01:42 + s01],
                                     lhsT=owt[:], rhs=oc[:],
                                     start=True, stop=True)
                    nc.vector.tensor_copy(ha[0:D, s:s + 1],
                                          misc[0:64, 41 + s01:42 + s01])

            # ---- batched tail over the 4 samples ----
            tl = ps.tile([128, 128], F32, tag="stats", bufs=2)
            eop = ps.tile([SPC, E * D], F32, tag="convA", bufs=2)
            nc.tensor.matmul(tl[0:SPC, 0:E], lhsT=ha[0:D, :], rhs=rwt[:],
                             start=True, stop=True)
            el = sm.tile([SPC, E], F32, tag="el")
            nc.scalar.activation(el[:], tl[0:SPC, 0:E], AF.Exp)
            zr = sm.tile([SPC, 1], F32, tag="zr")
            nc.vector.tensor_reduce(zr[:], el[:], mybir.AxisListType.X, OP.add)
            rr = sm.tile([SPC, 1], F32, tag="rr")
            nc.vector.reciprocal(rr[:], zr[:])
            rw = sm.tile([SPC, E], F32, tag="rw")
            nc.vector.tensor_scalar(rw[:], el[:], rr[:], None, op0=OP.mult)
            m8 = sm.tile([SPC, 8], F32, tag="m8")
            nc.vector.max(m8[:], rw[:])
            msk = sm.tile([SPC, E], F32, tag="msk")
            nc.vector.tensor_scalar(msk[:], rw[:], m8[:, TOPK - 1:TOPK], None,
                                    op0=OP.is_ge)
            w4 = sm.tile([SPC, E], F32, tag="w4")
            nc.vector.tensor_mul(w4[:], rw[:], msk[:])
            # expert outputs (dense) and weighted sum over selected experts
            nc.tensor.matmul(eop[:], lhsT=ha[:], rhs=wexp[:],
                             start=True, stop=True)
            prod = sm.tile([SPC, E * D], F32, tag="prod")
            nc.vector.tensor_tensor(
                prod[:].rearrange("p (e d) -> p e d", e=E), eop[:].rearrange("p (e d) -> p e d", e=E),
                w4[:].to_broadcast([SPC, E, D]), op=OP.mult)
            s1 = sm.tile([SPC, 256], F32, tag="s1")
            nc.vector.tensor_add(s1[:], prod[:, 0:256], prod[:, 256:512])
            s2 = sm.tile([SPC, 128], F32, tag="s2")
            nc.vector.tensor_add(s2[:], s1[:, 0:128], s1[:, 128:256])
            moe4 = sm.tile([SPC, D], F32, tag="moe4")
            nc.vector.tensor_add(moe4[:], s2[:, 0:64], s2[:, 64:128])
            # transpose to (64, 4), project through moe_out_w
            nc.tensor.transpose(tl[0:D, 8:8 + SPC], moe4[:], id4[:])
            moet = sm.tile([D, SPC], F32, tag="moet")
            nc.vector.tensor_copy(moet[:], tl[0:D, 8:8 + SPC])
            nc.tensor.matmul(tl[0:D, 16:16 + SPC], lhsT=mowt[:], rhs=moet[:],
                             start=True, stop=True)
            hm = sm.tile([D, SPC], F32, tag="hm")
            nc.vector.tensor_copy(hm[:], tl[0:D, 16:16 + SPC])
            # LN2
            nc.tensor.matmul(tl[0:1, 24:24 + SPC], lhsT=ones128[0:D, :],
                             rhs=hm[:], start=True, stop=True)
            mu2 = sm.tile([1, SPC], F32, tag="mu2")
            nc.scalar.activation(mu2[:], tl[0:1, 24:24 + SPC], AF.Copy,
                                 scale=1.0 / D)
            nc.tensor.matmul(tl[0:D, 28:28 + SPC], lhsT=onesr[0:1, 0:D],
                             rhs=mu2[:], start=True, stop=True)
            hc = sm.tile([D, SPC], F32, tag="hc")
            nc.vector.tensor_sub(hc[:], hm[:], tl[0:D, 28:28 + SPC])
            sq2 = sm.tile([D, SPC], F32, tag="sq2")
            nc.scalar.activation(sq2[:], hc[:], AF.Square)
            nc.tensor.matmul(tl[0:1, 24 + SPC:24 + 2 * SPC],
                             lhsT=ones128[0:D, :], rhs=sq2[:],
                             start=True, stop=True)
            var2 = sm.tile([1, SPC], F32, tag="var2")
            nc.scalar.activation(var2[:], tl[0:1, 24 + SPC:24 + 2 * SPC],
                                 AF.Copy, scale=1.0 / D)
            std2 = sm.tile([1, SPC], F32, tag="std2")
            nc.scalar.activation(std2[:], var2[:], AF.Sqrt, bias=epsb[0:1, :])
            rstd2 = sm.tile([1, SPC], F32, tag="rstd2")
            nc.vector.reciprocal(rstd2[:], std2[:])
            nc.tensor.matmul(tl[0:SPC, 40:41], lhsT=rstd2[:],
                             rhs=onesr[0:1, 0:1], start=True, stop=True)
            rsc = sm.tile([SPC, 1], F32, tag="rsc")
            nc.vector.tensor_copy(rsc[:], tl[0:SPC, 40:41])
            # final projection, scaled by rstd2 per row on eviction
            nc.tensor.matmul(tl[0:SPC, 32:32 + PRED], lhsT=hc[:], rhs=outwt[:],
                             start=True, stop=True)
            outp = sm.tile([SPC, PRED], F32, tag="outp")
            nc.scalar.activation(outp[:], tl[0:SPC, 32:32 + PRED], AF.Copy,
                                 scale=rsc[:])
            nc.sync.dma_start(Yout.ap(), outp[:])

    nc.compile()
    return nc


_NC_CACHE = {}


def _get_nc():
    if "nc" not in _NC_CACHE:
        _NC_CACHE["nc"] = build_nc()
    return _NC_CACHE["nc"]


def _prep_in_maps(inputs):
    f32 = np.float32
    X = np.ascontiguousarray(inputs["X"], f32)
    conv_w = np.asarray(inputs["conv_w"], f32)
    conv_b = np.asarray(inputs["conv_b"], f32)
    qw, kw, vw, ow = (np.asarray(inputs[k], f32) for k in ("qw", "kw", "vw", "ow"))
    expert_w = np.asarray(inputs["expert_w"], f32)
    expert_b = np.asarray(inputs["expert_b"], f32)
    router_w = np.asarray(inputs["router_w"], f32)
    moe_out_w = np.asarray(inputs["moe_out_w"], f32)
    out_w = np.asarray(inputs["out_w"], f32)

    np_x = mybir.dt.np(DT_X)
    Wc = np.ascontiguousarray(conv_w.transpose(2, 1, 0).reshape(C * P, D)).astype(np_x)
    pebT = (_pos_encoding_np(N, D) + conv_b[None, :]).T.astype(f32)  # (64, N)
    PEBT2 = np.ascontiguousarray(np.concatenate([pebT, pebT], axis=0))
    dbl = lambda a: np.ascontiguousarray(np.concatenate([a, a], axis=0), dtype=f32)
    QwT2 = dbl(qw.T)
    Kw2 = dbl(kw)
    VwT2 = dbl(vw.T)
    SqCol2 = dbl(qw.sum(1)[:, None])
    SkCol2 = dbl(kw.sum(1)[:, None])
    SvCol = np.ascontiguousarray(vw.sum(1)[:, None], dtype=f32)
    SelAB = np.zeros((128, 2), f32)
    SelAB[0:64, 0] = 1.0
    SelAB[64:128, 1] = 1.0
    OnesRow = np.ones((1, 128), f32)
    Ones128 = np.ones((128, 1), f32)
    OneHot = np.zeros((128, 1), f32)
    OneHot[126, 0] = 1.0
    LastMask = np.ones((128, 1), f32)
    LastMask[127, 0] = 0.0
    WexpE = np.concatenate(
        [expert_w.transpose(2, 0, 1).reshape(D, E * D),
         expert_b.reshape(1, E * D)], axis=0).astype(f32)
    common = dict(
        Wc=Wc, PEBT2=PEBT2, QwT2=QwT2, Kw2=Kw2, VwT2=VwT2,
        OwT=np.ascontiguousarray(ow.T), SqCol2=SqCol2, SkCol2=SkCol2,
        SvCol=SvCol, SelAB=SelAB, OnesRow=OnesRow, Ones128=Ones128,
        OneHot=OneHot, LastMask=LastMask,
        RwT=np.ascontiguousarray(router_w.T),
        WexpE=np.ascontiguousarray(WexpE),
        MowT=np.ascontiguousarray(moe_out_w.T),
        OutWT=np.ascontiguousarray(out_w.T),
        Id4=np.eye(SPC, dtype=f32),
    )
    common = {k: np.ascontiguousarray(v, dtype=f32) for k, v in common.items()}
    common["Wc"] = Wc
    in_maps = []
    for c in range(NCORES):
        m = dict(common)
        m["Xs"] = np.ascontiguousarray(X[c * SPC:(c + 1) * SPC]).astype(np_x)
        in_maps.append(m)
    return in_maps


def kernel(**inputs) -> np.ndarray:
    nc = _get_nc()
    in_maps = _prep_in_maps(inputs)
    res = run_bass_kernel_spmd(nc, in_maps, core_ids=list(range(NCORES)))
    out = np.concatenate([res.results[c]["Yout"] for c in range(NCORES)], axis=0)
    return out.astype(np.float32)



# revision 7
# speedup vs baseline: 1.8645x; 1.8645x over previous
"""Trainium2 Bass kernel for nn_Decoder_22703197127089 (moe_routing).

Key insight: the module's output depends only on each sample's LAST token
(h[:, -1, :] is taken after the MoE block), so the MoE block and all
attention rows except the last are dead code.  What remains per sample:
  conv1d patch embed (all 1023 tokens) -> LN1 -> scores/v for the last
  attention row (rank-1 tricks fold LN into the projections) -> out-proj
  -> MoE for 1 token -> LN2 -> final linear (96).

Perf structure (cost-model driven):
  - conv runs in fp8e4 DoubleRow mode (K=128 per pass via the dim1=2
    subtile trick: subtile j = patch position 2k+j), 0.5 cycles/row.
    X and conv_w are quantized to fp8 on host; conv_w is pre-scaled by
    64 so its values sit in e4m3's normal range.  h0 is then 64*h0_true;
    LN makes everything downstream scale-invariant (verified: final
    rel err ~1.5e-2 < 2e-2 budget).
  - all small constants ride in ONE f32 DMA + one bf16 DMA (pebt +
    expert weights); X DMAs are ordered so PE never starves.
  - every activation uses funcs from the natural_log_exp_and_others
    table (ln/exp/square/copy): rstd = exp(-0.5*ln(var+eps)) replaces
    Sqrt+reciprocal, so only one act-table load happens.
  - PE order: warmup (p-state ramp) -> all 4 samples' conv -> both
    pairs' attention chains interleaved (generators) -> batched tail.

Sharding: data-parallel over batch B=32 across 8 cores (4 samples/core).
No collectives; host gathers the (4, 96) per-core outputs.
"""

import numpy as np

import concourse.bass as bass
import concourse.mybir as mybir
import concourse.tile as tile
from concourse import bacc
from concourse.bass_utils import run_bass_kernel_spmd

F32 = mybir.dt.float32
BF16 = mybir.dt.bfloat16
F8 = mybir.dt.float8e4
AF = mybir.ActivationFunctionType
OP = mybir.AluOpType
DR = mybir.MatmulPerfMode.DoubleRow

B, C, L = 32, 64, 12288
D = 64
E = 8
TOPK = 4
P, S = 24, 12
PRED = 96
N = (L - P) // S + 1  # 1023
NT = 1024             # padded token dim (col 1023 zeroed)
NJ = 8                # 128-token chunks
EPS = 1e-5
NCORES = 8
SPC = B // NCORES     # 4 samples per core
NPAIR = SPC // 2      # 2
NCH = 12              # DoubleRow contraction chunks: (2 positions x 64 ch)
WSC = 64.0            # fp8 weight pre-scale (cancels through LN)
NWARM = 11            # PE warmup matmuls (p-state ramp bridge)
XSPLIT = 6156         # X column split: tokens 0..511 need cols < 6156

# conv m-chunks: (psum tile idx, psum col, token0, ntok)
QCHUNKS = [(0, 0, 0, 256), (0, 256, 256, 256),
           (1, 0, 512, 256), (1, 256, 768, 255)]

# CB (f32 const block) column offsets
CB_SQCOL = 0      # qw.sum(1) doubled           (128,1)
CB_SKCOL = 1      # kw.sum(1) doubled           (128,1)
CB_SVCOL = 2      # vw.sum(1)                   (64,1)
CB_SELAB = 3      # [[1;0],[0;1]] selector      (128,2)
CB_ONES = 5       # ones column                 (128,1)
CB_ONEHOT = 6     # 1.0 at partition 126        (128,1)
CB_LASTM = 7      # ones, 0.0 at partition 127  (128,1)
CB_RWT = 8        # router_w.T                  (64,8)
CB_QWT = 16       # qw.T doubled                (128,64)
CB_KW = 80        # kw doubled                  (128,64)
CB_VWT = 144      # vw.T doubled                (128,64)
CB_OWT = 208      # ow.T                        (64,64)
CB_MOWT = 272     # moe_out_w.T                 (64,64)
CB_OUTWT = 336    # out_w.T                     (64,96)
CB_ID4 = 432      # eye(4)                      (4,4)
CB_ONESR = 436    # row 0 = ones                (1,128)
CB_W = 564


def _pos_encoding_np(n, d):
    pos = np.arange(n, dtype=np.float32)[:, None]
    div = np.exp(np.arange(0, d, 2, dtype=np.float32)
                 * (np.float32(-np.log(np.float32(10000.0))) / np.float32(d)))
    pe = np.zeros((n, d), np.float32)
    pe[:, 0::2] = np.sin(pos * div)
    pe[:, 1::2] = np.cos(pos * div)
    return pe


def build_nc():
    nc = bacc.Bacc("TRN2", target_bir_lowering=False, debug=False,
                   num_devices=NCORES)

    Xs = nc.dram_tensor("Xs", [SPC, C, L], F8, kind="ExternalInput")
    W8 = nc.dram_tensor("W8", [C, NCH * 2 * D], F8, kind="ExternalInput")
    PEBTC = nc.dram_tensor("PEBTC", [128, NT + E * D], BF16,
                           kind="ExternalInput")
    CB = nc.dram_tensor("CB", [128, CB_W], F32, kind="ExternalInput")
    Yout = nc.dram_tensor("Yout", [SPC, PRED], F32, kind="ExternalOutput")

    with tile.TileContext(nc) as tc:
        with (
            tc.tile_pool(name="const", bufs=1) as pc,
            tc.tile_pool(name="hp", bufs=2) as hp,
            tc.tile_pool(name="sqp", bufs=2) as sqp,
            tc.tile_pool(name="vp", bufs=2) as vp,
            tc.tile_pool(name="sm", bufs=2) as sm,
            tc.tile_pool(name="ps", bufs=2, space="PSUM") as ps,
        ):
            # ---- SBUF tiles & DMA order (DMA_ENGINES serializes in this
            # order; arrange so PE conv never starves) ----
            w8 = pc.tile([C, NCH * 2 * D], F8, tag="w8")
            x8 = [pc.tile([C, L], F8, tag=f"x8_{s}", name=f"x8_{s}")
                  for s in range(SPC)]
            pebtc = pc.tile([128, NT + E * D], BF16, tag="pebtc")
            cb = pc.tile([128, CB_W], F32, tag="cb")

            nc.sync.dma_start(w8[:], W8.ap())
            nc.sync.dma_start(x8[0][:, 0:XSPLIT], Xs.ap()[0][:, 0:XSPLIT])
            nc.sync.dma_start(x8[0][:, XSPLIT:L], Xs.ap()[0][:, XSPLIT:L])
            nc.sync.dma_start(pebtc[:], PEBTC.ap())
            nc.sync.dma_start(x8[1][:], Xs.ap()[1])
            nc.sync.dma_start(x8[2][:], Xs.ap()[2])
            nc.sync.dma_start(x8[3][:], Xs.ap()[3])
            nc.sync.dma_start(cb[:], CB.ap())

            pebt = pebtc[:, 0:NT]
            wexpb = pebtc[0:D + 1, NT:NT + E * D]
            sqcol = cb[:, CB_SQCOL:CB_SQCOL + 1]
            skcol = cb[:, CB_SKCOL:CB_SKCOL + 1]
            svcol = cb[0:D, CB_SVCOL:CB_SVCOL + 1]
            selab = cb[:, CB_SELAB:CB_SELAB + 2]
            ones128 = cb[:, CB_ONES:CB_ONES + 1]
            onehot = cb[:, CB_ONEHOT:CB_ONEHOT + 1]
            lastm = cb[:, CB_LASTM:CB_LASTM + 1]
            rwt = cb[0:D, CB_RWT:CB_RWT + E]
            qwt = cb[:, CB_QWT:CB_QWT + D]
            kw2 = cb[:, CB_KW:CB_KW + D]
            vwt = cb[:, CB_VWT:CB_VWT + D]
            owt = cb[0:D, CB_OWT:CB_OWT + D]
            mowt = cb[0:D, CB_MOWT:CB_MOWT + D]
            outwt = cb[0:D, CB_OUTWT:CB_OUTWT + PRED]
            id4 = cb[0:SPC, CB_ID4:CB_ID4 + SPC]
            onesr = cb[0:1, CB_ONESR:CB_ONESR + 128]

            ha = pc.tile([D + 1, SPC], F32, tag="ha")
            nc.vector.memset(ha[D:D + 1, :], 1.0)
            epsb = pc.tile([128, 1], F32, tag="epsb")
            nc.vector.memset(epsb[:], EPS)

            # ---- PE warmup: ramp the p-state while DMAs stream ----
            dum = pc.tile([C, 512], BF16, tag="dum")
            nc.vector.memset(dum[:], 0.0)
            warm = ps.tile([C, 512], F32, tag="cps", name="warm")
            for i in range(NWARM):
                nc.tensor.matmul(warm[0:C, 0:512], lhsT=dum[:, 0:C],
                                 rhs=dum[:], start=True, stop=True)
            wsink = sm.tile([C, 1], F32, tag="wsink")
            nc.vector.tensor_copy(wsink[:], warm[0:C, 0:1])

            # ---- conv: all 4 samples back-to-back on PE ----
            w8v = w8[:].rearrange("p (k j d) -> p k j d", k=NCH, j=2)
            h0s, sqs = [], []
            for pair in range(NPAIR):
                h0 = hp.tile([128, NT], F32, tag="h0", name=f"h0_{pair}")
                nc.vector.memset(h0[:, N:NT], 0.0)
                h0s.append(h0)
                sqs.append(sqp.tile([128, NT], F32, tag="sq",
                                    name=f"sq_{pair}"))
            for s in range(SPC):
                pair, s01 = divmod(s, 2)
                o = C * s01
                xv = x8[s][:].rearrange("p (n t) -> p t n", t=S)
                cps = None
                for pi, c0, n0, nn in QCHUNKS:
                    if c0 == 0:
                        cps = ps.tile([C, 512], F32, tag="cps",
                                      name=f"cps{s}{pi}")
                    for k in range(NCH):
                        q, r = divmod(2 * k, S)
                        nc.tensor.matmul(
                            cps[0:C, c0:c0 + nn],
                            lhsT=w8v[:, k],
                            rhs=xv[:, r:r + 2, n0 + q:n0 + q + nn],
                            start=(k == 0), stop=(k == NCH - 1),
                            perf_mode=DR)
                    if c0 != 0:
                        # evict sample psum -> its h0 half (cross-partition
                        # for the pair's second sample), adding pe+bias
                        w = c0 + nn
                        nc.vector.tensor_add(
                            h0s[pair][o:o + C, 512 * pi:512 * pi + w],
                            cps[0:C, 0:w],
                            pebt[o:o + C, 512 * pi:512 * pi + w])

            # ---- attention: both pairs' chains interleaved ----
            def attn_gen(pair):
                h0, sq = h0s[pair], sqs[pair]
                # LN1 per-token stats via PE selector matmuls
                nc.scalar.activation(sq[:], h0[:], AF.Square)
                yield
                stp = ps.tile([128, 128], F32, tag="stp",
                              name=f"stp{pair}")
                for j in range(NJ):
                    nc.tensor.matmul(stp[:, 2 * j:2 * j + 2],
                                     lhsT=h0[:, 128 * j:128 * j + 128],
                                     rhs=selab, start=True, stop=True)
                    nc.tensor.matmul(stp[:, 16 + 2 * j:16 + 2 * j + 2],
                                     lhsT=sq[:, 128 * j:128 * j + 128],
                                     rhs=selab, start=True, stop=True)
                yield
                mean = sm.tile([128, 16], F32, tag="mean")
                nc.vector.tensor_scalar_mul(mean[:], stp[:, 0:16], 1.0 / D)
                ex2 = sm.tile([128, 16], F32, tag="ex2")
                nc.vector.tensor_scalar_mul(ex2[:], stp[:, 16:32], 1.0 / D)
                var = sm.tile([128, 16], F32, tag="var")
                nc.vector.tensor_mul(var[:], mean[:], mean[:])
                nc.vector.tensor_sub(var[:], ex2[:], var[:])
                yield
                # rstd = exp(-0.5*ln(var+eps)); r8 folds the 1/sqrt(D)
                lnv = sm.tile([128, 16], F32, tag="lnv")
                nc.scalar.activation(lnv[:], var[:], AF.Ln, bias=epsb[:])
                rstd = sm.tile([128, 16], F32, tag="rstd")
                nc.scalar.activation(rstd[:], lnv[:], AF.Exp, scale=-0.5)
                r8 = sm.tile([128, 16], F32, tag="r8")
                nc.vector.tensor_scalar_mul(r8[:], rstd[:], 0.125)
                yield
                # q for the last token + (mu,r) of the last token
                misc = ps.tile([128, 44], F32, tag="misc",
                               name=f"misc{pair}")
                qe = sm.tile([128, 1], F32, tag="qe")
                for s01 in range(2):
                    o = C * s01
                    nc.tensor.matmul(misc[o:o + C, 0:1], lhsT=qwt[o:o + C, :],
                                     rhs=h0[o:o + C, N - 1:N],
                                     start=True, stop=True)
                    nc.tensor.matmul(misc[0:1, 20 + 2 * s01:21 + 2 * s01],
                                     lhsT=mean[:, 14 + s01:15 + s01],
                                     rhs=onehot, start=True, stop=True)
                    nc.tensor.matmul(misc[0:1, 21 + 2 * s01:22 + 2 * s01],
                                     lhsT=rstd[:, 14 + s01:15 + s01],
                                     rhs=onehot, start=True, stop=True)
                yield
                ex4 = sm.tile([1, 4], F32, tag="ex4")
                nc.vector.tensor_copy(ex4[:], misc[0:1, 20:24])
                yield
                for s01 in range(2):
                    o = C * s01
                    nc.tensor.matmul(misc[o:o + C, 24:25],
                                     lhsT=onesr[0:1, 0:C],
                                     rhs=ex4[0:1, 2 * s01:2 * s01 + 1],
                                     start=True, stop=True)
                    nc.tensor.matmul(misc[o:o + C, 25:26],
                                     lhsT=onesr[0:1, 0:C],
                                     rhs=ex4[0:1, 2 * s01 + 1:2 * s01 + 2],
                                     start=True, stop=True)
                yield
                for s01 in range(2):
                    o = C * s01
                    # q_eff = r_last * (q0 - mu_last * Sq)
                    nc.vector.tensor_mul(qe[o:o + C, :], sqcol[o:o + C, :],
                                         misc[o:o + C, 24:25])
                    nc.vector.tensor_sub(qe[o:o + C, :], misc[o:o + C, 0:1],
                                         qe[o:o + C, :])
                    nc.vector.tensor_mul(qe[o:o + C, :], qe[o:o + C, :],
                                         misc[o:o + C, 25:26])
                yield
                for s01 in range(2):
                    o = C * s01
                    nc.tensor.matmul(misc[0:1, 26 + s01:27 + s01],
                                     lhsT=qe[o:o + C, :],
                                     rhs=skcol[o:o + C, :],
                                     start=True, stop=True)
                    nc.tensor.matmul(misc[o:o + C, 1:2],
                                     lhsT=kw2[o:o + C, :],
                                     rhs=qe[o:o + C, :],
                                     start=True, stop=True)
                yield
                qks = sm.tile([128, 1], F32, tag="qks")
                nc.vector.tensor_copy(qks[:], misc[:, 1:2])
                c1r = sm.tile([1, 2], F32, tag="c1r")
                nc.vector.tensor_copy(c1r[:], misc[0:1, 26:28])
                yield
                nc.tensor.matmul(misc[:, 2:4], lhsT=onesr, rhs=c1r[:],
                                 start=True, stop=True)
                for j in range(NJ):
                    for s01 in range(2):
                        o = C * s01
                        nc.tensor.matmul(
                            misc[:, 4 + 2 * j + s01:5 + 2 * j + s01],
                            lhsT=h0[o:o + C, 128 * j:128 * j + 128],
                            rhs=qks[o:o + C, :], start=True, stop=True)
                yield
                mv3 = mean[:].rearrange("p (j t) -> p j t", t=2)
                tmp = sm.tile([128, 16], F32, tag="tmp")
                nc.vector.tensor_tensor(
                    tmp[:].rearrange("p (j t) -> p j t", t=2), mv3,
                    misc[:, 2:4][:, None].to_broadcast([128, NJ, 2]),
                    op=OP.mult)
                sc = sm.tile([128, 16], F32, tag="sc")
                nc.vector.tensor_sub(sc[:], misc[:, 4:20], tmp[:])
                nc.vector.tensor_mul(sc[:], sc[:], r8[:])
                yield
                exps = sm.tile([128, 16], F32, tag="exps")
                nc.scalar.activation(exps[:], sc[:], AF.Exp)
                nc.vector.tensor_scalar(exps[:, 14:16], exps[:, 14:16],
                                        lastm[:], None, op0=OP.mult)
                yield
                # Z per sample: reduce over chunks, then over partitions
                zs = sm.tile([128, 2], F32, tag="zs")
                nc.vector.tensor_reduce(
                    zs[:], exps[:].rearrange("p (j t) -> p t j", t=2),
                    mybir.AxisListType.X, OP.add)
                yield
                nc.tensor.matmul(misc[0:2, 28:29], lhsT=zs[:], rhs=ones128,
                                 start=True, stop=True)
                yield
                z2s = sm.tile([2, 1], F32, tag="z2s")
                nc.vector.tensor_copy(z2s[:], misc[0:2, 28:29])
                yield
                nc.tensor.matmul(misc[0:1, 29:31], lhsT=z2s[:],
                                 rhs=id4[0:2, 0:2], start=True, stop=True)
                yield
                rzr = sm.tile([1, 2], F32, tag="rzr")
                nc.vector.reciprocal(rzr[:], misc[0:1, 29:31])
                yield
                nc.tensor.matmul(misc[:, 32:34], lhsT=onesr, rhs=rzr[:],
                                 start=True, stop=True)
                yield
                # wr = exps/Z * rstd ; g = sum(wr * mu) per sample
                wr = sm.tile([128, 16], F32, tag="wr")
                nc.vector.tensor_tensor(
                    wr[:].rearrange("p (j t) -> p j t", t=2),
                    exps[:].rearrange("p (j t) -> p j t", t=2),
                    misc[:, 32:34][:, None].to_broadcast([128, NJ, 2]),
                    op=OP.mult)
                nc.vector.tensor_mul(wr[:], wr[:], rstd[:])
                gt = sm.tile([128, 16], F32, tag="gt")
                nc.vector.tensor_mul(gt[:], wr[:], mean[:])
                gs = sm.tile([128, 2], F32, tag="gs")
                nc.vector.tensor_reduce(
                    gs[:], gt[:].rearrange("p (j t) -> p t j", t=2),
                    mybir.AxisListType.X, OP.add)
                yield
                nc.tensor.matmul(misc[0:2, 34:35], lhsT=gs[:], rhs=ones128,
                                 start=True, stop=True)
                yield
                g2s = sm.tile([2, 1], F32, tag="g2s")
                nc.vector.tensor_copy(g2s[:], misc[0:2, 34:35])
                yield
                nc.tensor.matmul(misc[0:1, 35:37], lhsT=g2s[:],
                                 rhs=id4[0:2, 0:2], start=True, stop=True)
                yield
                grow = sm.tile([1, 2], F32, tag="grow")
                nc.vector.tensor_copy(grow[:], misc[0:1, 35:37])
                yield
                nc.tensor.matmul(misc[0:C, 37:39], lhsT=onesr[0:1, 0:C],
                                 rhs=grow[:], start=True, stop=True)
                yield
                # v0 + attention accumulate + out-proj, per sample
                for s01 in range(2):
                    s = 2 * pair + s01
                    o = C * s01
                    vps = ps.tile([128, 512], F32, tag="vps",
                                  name=f"vps{s}")
                    for j in range(NJ):
                        nc.tensor.matmul(vps[:, 64 * j:64 * j + 64],
                                         lhsT=h0[o:o + C, 128 * j:128 * j + 128],
                                         rhs=vwt[o:o + C, :],
                                         start=True, stop=True)
                    v0 = vp.tile([128, 512], F32, tag="v0sb")
                    nc.scalar.copy(v0[:], vps[:])
                    yield
                    for j in range(NJ):
                        nc.tensor.matmul(
                            misc[0:C, 39 + s01:40 + s01],
                            lhsT=v0[:, 64 * j:64 * j + 64],
                            rhs=wr[:, 2 * j + s01:2 * j + s01 + 1],
                            start=(j == 0), stop=(j == NJ - 1))
                    yield
                    oc = sm.tile([D, 1], F32, tag="oc")
                    nc.vector.tensor_mul(oc[:], svcol,
                                         misc[0:C, 37 + s01:38 + s01])
                    nc.vector.tensor_sub(oc[:], misc[0:C, 39 + s01:40 + s01],
                                         oc[:])
                    yield
                    nc.tensor.matmul(misc[0:C, 41 + s01:42 + s01],
                                     lhsT=owt, rhs=oc[:],
                                     start=True, stop=True)
                    yield
                    nc.vector.tensor_copy(ha[0:D, s:s + 1],
                                          misc[0:C, 41 + s01:42 + s01])
                    yield

            gens = [attn_gen(0), attn_gen(1)]
            alive = list(gens)
            while alive:
                for g in list(alive):
                    try:
                        next(g)
                    except StopIteration:
                        alive.remove(g)

            # ---- batched tail over the 4 samples ----
            tl = ps.tile([128, 128], F32, tag="stp", name="tl")
            eop = ps.tile([C, 512], F32, tag="cps", name="eop")
            nc.tensor.matmul(tl[0:SPC, 0:E], lhsT=ha[0:D, :], rhs=rwt,
                             start=True, stop=True)
            el = sm.tile([SPC, E], F32, tag="el")
            nc.scalar.activation(el[:], tl[0:SPC, 0:E], AF.Exp)
            zr = sm.tile([SPC, 1], F32, tag="zr")
            nc.vector.tensor_reduce(zr[:], el[:], mybir.AxisListType.X, OP.add)
            rr = sm.tile([SPC, 1], F32, tag="rr")
            nc.vector.reciprocal(rr[:], zr[:])
            rw = sm.tile([SPC, E], F32, tag="rw")
            nc.vector.tensor_scalar(rw[:], el[:], rr[:], None, op0=OP.mult)
            m8 = sm.tile([SPC, 8], F32, tag="m8")
            nc.vector.max(m8[:], rw[:])
            msk = sm.tile([SPC, E], F32, tag="msk")
            nc.vector.tensor_scalar(msk[:], rw[:], m8[:, TOPK - 1:TOPK], None,
                                    op0=OP.is_ge)
            w4 = sm.tile([SPC, E], F32, tag="w4")
            nc.vector.tensor_mul(w4[:], rw[:], msk[:])
            # expert outputs (dense, bf16) + weighted sum over selected
            hab = sm.tile([D + 1, SPC], BF16, tag="hab")
            nc.vector.tensor_copy(hab[:], ha[:])
            nc.tensor.matmul(eop[0:SPC, 0:E * D], lhsT=hab[:], rhs=wexpb,
                             start=True, stop=True)
            prod = sm.tile([SPC, E * D], F32, tag="prod")
            nc.vector.tensor_tensor(
                prod[:].rearrange("p (e d) -> p e d", e=E),
                eop[0:SPC, 0:E * D].rearrange("p (e d) -> p e d", e=E),
                w4[:].to_broadcast([SPC, E, D]), op=OP.mult)
            s1 = sm.tile([SPC, 256], F32, tag="s1")
            nc.vector.tensor_add(s1[:], prod[:, 0:256], prod[:, 256:512])
            s2 = sm.tile([SPC, 128], F32, tag="s2")
            nc.vector.tensor_add(s2[:], s1[:, 0:128], s1[:, 128:256])
            moe4 = sm.tile([SPC, D], F32, tag="moe4")
            nc.vector.tensor_add(moe4[:], s2[:, 0:64], s2[:, 64:128])
            # transpose to (64, 4), project through moe_out_w
            nc.tensor.transpose(tl[0:D, 8:8 + SPC], moe4[:], id4)
            moet = sm.tile([D, SPC], F32, tag="moet")
            nc.vector.tensor_copy(moet[:], tl[0:D, 8:8 + SPC])
            nc.tensor.matmul(tl[0:D, 16:16 + SPC], lhsT=mowt, rhs=moet[:],
                             start=True, stop=True)
            hm = sm.tile([D, SPC], F32, tag="hm")
            nc.vector.tensor_copy(hm[:], tl[0:D, 16:16 + SPC])
            # LN2
            nc.tensor.matmul(tl[0:1, 24:24 + SPC], lhsT=ones128[0:D, :],
                             rhs=hm[:], start=True, stop=True)
            mu2 = sm.tile([1, SPC], F32, tag="mu2")
            nc.scalar.activation(mu2[:], tl[0:1, 24:24 + SPC], AF.Copy,
                                 scale=1.0 / D)
            nc.tensor.matmul(tl[0:D, 28:28 + SPC], lhsT=onesr[0:1, 0:D],
                             rhs=mu2[:], start=True, stop=True)
            hc = sm.tile([D, SPC], F32, tag="hc")
            nc.vector.tensor_sub(hc[:], hm[:], tl[0:D, 28:28 + SPC])
            sq2 = sm.tile([D, SPC], F32, tag="sq2")
            nc.scalar.activation(sq2[:], hc[:], AF.Square)
            nc.tensor.matmul(tl[0:1, 24 + SPC:24 + 2 * SPC],
                             lhsT=ones128[0:D, :], rhs=sq2[:],
                             start=True, stop=True)
            var2 = sm.tile([1, SPC], F32, tag="var2")
            nc.scalar.activation(var2[:], tl[0:1, 24 + SPC:24 + 2 * SPC],
                                 AF.Copy, scale=1.0 / D)
            lnv2 = sm.tile([1, SPC], F32, tag="lnv2")
            nc.scalar.activation(lnv2[:], var2[:], AF.Ln, bias=epsb[0:1, :])
            rstd2 = sm.tile([1, SPC], F32, tag="rstd2")
            nc.scalar.activation(rstd2[:], lnv2[:], AF.Exp, scale=-0.5)
            nc.tensor.matmul(tl[0:SPC, 40:41], lhsT=rstd2[:],
                             rhs=onesr[0:1, 0:1], start=True, stop=True)
            rsc = sm.tile([SPC, 1], F32, tag="rsc")
            nc.vector.tensor_copy(rsc[:], tl[0:SPC, 40:41])
            # final projection, scaled by rstd2 per row on eviction
            nc.tensor.matmul(tl[0:SPC, 32:32 + PRED], lhsT=hc[:], rhs=outwt,
                             start=True, stop=True)
            outp = sm.tile([SPC, PRED], F32, tag="outp")
            nc.scalar.activation(outp[:], tl[0:SPC, 32:32 + PRED], AF.Copy,
                                 scale=rsc[:])
            nc.sync.dma_start(Yout.ap(), outp[:])

    nc.compile()
    return nc


_NC_CACHE = {}


def _get_nc():
    if "nc" not in _NC_CACHE:
        _NC_CACHE["nc"] = build_nc()
    return _NC_CACHE["nc"]


def _prep_in_maps(inputs):
    f32 = np.float32
    np_f8 = mybir.dt.np(F8)
    np_bf = mybir.dt.np(BF16)
    X = np.ascontiguousarray(inputs["X"], f32)
    conv_w = np.asarray(inputs["conv_w"], f32)
    conv_b = np.asarray(inputs["conv_b"], f32)
    qw, kw, vw, ow = (np.asarray(inputs[k], f32)
                      for k in ("qw", "kw", "vw", "ow"))
    expert_w = np.asarray(inputs["expert_w"], f32)
    expert_b = np.asarray(inputs["expert_b"], f32)
    router_w = np.asarray(inputs["router_w"], f32)
    moe_out_w = np.asarray(inputs["moe_out_w"], f32)
    out_w = np.asarray(inputs["out_w"], f32)

    # conv weights: [c, (k, j, d)] = conv_w[d, c, 2k+j] * WSC, fp8
    W8 = np.ascontiguousarray(
        conv_w.transpose(1, 2, 0).reshape(C, P * D) * WSC
    ).astype(np_f8)

    # bf16 block: pebt*WSC (doubled rows, col 1023 zero) + expert weights
    pebT = ((_pos_encoding_np(N, D) + conv_b[None, :]) * WSC).T.astype(f32)
    PEBTC = np.zeros((128, NT + E * D), f32)
    PEBTC[0:D, 0:N] = pebT
    PEBTC[D:128, 0:N] = pebT
    wexp = np.concatenate(
        [expert_w.transpose(2, 0, 1).reshape(D, E * D),
         expert_b.reshape(1, E * D)], axis=0)
    PEBTC[0:D + 1, NT:NT + E * D] = wexp
    PEBTC = PEBTC.astype(np_bf)

    CBa = np.zeros((128, CB_W), f32)
    dbl = lambda a: np.concatenate([a, a], axis=0)
    CBa[:, CB_SQCOL] = dbl(qw.sum(1))
    CBa[:, CB_SKCOL] = dbl(kw.sum(1))
    CBa[0:D, CB_SVCOL] = vw.sum(1)
    CBa[0:D, CB_SELAB] = 1.0
    CBa[D:128, CB_SELAB + 1] = 1.0
    CBa[:, CB_ONES] = 1.0
    CBa[126, CB_ONEHOT] = 1.0
    CBa[:, CB_LASTM] = 1.0
    CBa[127, CB_LASTM] = 0.0
    CBa[0:D, CB_RWT:CB_RWT + E] = router_w.T
    CBa[:, CB_QWT:CB_QWT + D] = dbl(qw.T)
    CBa[:, CB_KW:CB_KW + D] = dbl(kw)
    CBa[:, CB_VWT:CB_VWT + D] = dbl(vw.T)
    CBa[0:D, CB_OWT:CB_OWT + D] = ow.T
    CBa[0:D, CB_MOWT:CB_MOWT + D] = moe_out_w.T
    CBa[0:D, CB_OUTWT:CB_OUTWT + PRED] = out_w.T
    CBa[0:SPC, CB_ID4:CB_ID4 + SPC] = np.eye(SPC, dtype=f32)
    CBa[0, CB_ONESR:CB_ONESR + 128] = 1.0

    common = dict(W8=W8, PEBTC=PEBTC, CB=np.ascontiguousarray(CBa))
    in_maps = []
    for c in range(NCORES):
        m = dict(common)
        m["Xs"] = np.ascontiguousarray(
            X[c * SPC:(c + 1) * SPC]).astype(np_f8)
        in_maps.append(m)
    return in_maps


def kernel(**inputs) -> np.ndarray:
    nc = _get_nc()
    in_maps = _prep_in_maps(inputs)
    res = run_bass_kernel_spmd(nc, in_maps, core_ids=list(range(NCORES)))
    out = np.concatenate([res.results[c]["Yout"] for c in range(NCORES)],
                         axis=0)
    return out.astype(np.float32)


# revision 10
# speedup vs baseline: 2.1477x; 1.1519x over previous
"""Trainium2 Bass kernel for nn_Decoder_22703197127089 (moe_routing).

Key insight: the module's output depends only on each sample's LAST token
(h[:, -1, :] is taken after the MoE block), so the MoE block and all
attention rows except the last are dead code.  What remains per sample:
  conv1d patch embed (all 1023 tokens) -> LN1 -> scores/v for the last
  attention row (rank-1 tricks fold LN into the projections) -> out-proj
  -> MoE for 1 token -> LN2 -> final linear (96).

Perf structure (cost-model driven):
  - conv runs in fp8e4 DoubleRow mode (K=128 per pass via the dim1=2
    subtile trick: subtile j = patch position 2k+j), 0.5 cycles/row.
    X and conv_w are quantized to fp8 on host; conv_w is pre-scaled by
    64 so its values sit in e4m3's normal range.  h0 is then 64*h0_true;
    LN makes everything downstream scale-invariant (final rel err
    ~1.5e-2 < 2e-2 budget).  DoubleRow dst must be psum partition 0, so
    each sample accumulates at base 0 and the pair-stack happens at
    evict time (cross-partition DVE write, verified legal).
  - all small constants ride in ONE f32 DMA + one bf16 DMA; X DMAs are
    ordered so PE never starves, and warmup/bridge matmuls keep the PE
    p-state at full clock (2x) through the conv.
  - one activation-table load total: get_activation_tables is patched
    (placement pass only) so every activation resolves to the
    natural_log_exp_and_others set; rstd = exp(-0.5*ln(var+eps))
    replaces Sqrt+reciprocal.
  - per-pair attention AND tail run as interleaved generators so the two
    chains overlap across engines; latency-critical reductions use
    single-matmul partition reduce (lhsT=ones).

Sharding: data-parallel over batch B=32 across 8 cores (4 samples/core).
No collectives; host gathers the (4, 96) per-core outputs.
"""

import math

import numpy as np

import concourse.bass as bass
import concourse.mybir as mybir
import concourse.tile as tile
from concourse import bacc
from concourse.bass_utils import run_bass_kernel_spmd

F32 = mybir.dt.float32
BF16 = mybir.dt.bfloat16
F8 = mybir.dt.float8e4
AF = mybir.ActivationFunctionType
OP = mybir.AluOpType
DR = mybir.MatmulPerfMode.DoubleRow

B, C, L = 32, 64, 12288
D = 64
E = 8
TOPK = 4
P, S = 24, 12
PRED = 96
N = (L - P) // S + 1  # 1023
NT = 1024             # padded token dim (col 1023 zeroed)
NJ = 8                # 128-token chunks
EPS = 1e-5
NCORES = 8
SPC = B // NCORES     # 4 samples per core
NPAIR = SPC // 2      # 2
NCH = 12              # DoubleRow contraction chunks: (2 positions x 64 ch)
WSC = 64.0            # fp8 weight pre-scale (cancels through LN)
NWARM = 12            # PE warmup matmuls before conv (p-state ramp)
NBRIDGE = 16          # PE bridge matmuls between sample 0 and 1
XSPLIT = 6156         # X column split: tokens 0..511 need cols < 6156

# conv m-chunks: (psum tile idx, psum col, token0, ntok)
QCHUNKS = [(0, 0, 0, 256), (0, 256, 256, 256),
           (1, 0, 512, 256), (1, 256, 768, 255)]

# CB (f32 const block) column offsets
CB_SQCOL = 0      # qw.sum(1) doubled           (128,1)
CB_SKCOL = 1      # kw.sum(1) doubled           (128,1)
CB_SVCOL = 2      # vw.sum(1)                   (64,1)
CB_SELAB = 3      # [[1;0],[0;1]] selector      (128,2)
CB_ONES = 5       # ones column                 (128,1)
CB_ONEHOT = 6     # 1.0 at partition 126        (128,1)
CB_LASTM = 7      # ones, 0.0 at partition 127  (128,1)
CB_RWT = 8        # router_w.T                  (64,8)
CB_QWT = 16       # qw.T doubled                (128,64)
CB_KW = 80        # kw doubled                  (128,64)
CB_OWT = 144      # ow.T                        (64,64)
CB_MOWT = 208     # moe_out_w.T                 (64,64)
CB_OUTWT = 272    # out_w.T                     (64,96)
CB_ID4 = 368      # eye(4)                      (4,4)
CB_ONESR = 372    # row 0 = ones                (1,128)
CB_W = 500

# PEBTC (bf16 const block) columns: [0:NT] pebt*WSC, [NT:NT+512] experts,
# [NT+512:NT+576] vw.T doubled
PB_WEXP = NT
PB_VWT = NT + E * D
PB_W = NT + E * D + D

_ACT_PATCHED = False


def _patch_act_tables():
    """Make the act-table placement pass resolve every activation to the
    natural_log_exp_and_others set (the only set holding both exp and
    ln), so exactly one table load is emitted.  Only bacc's placement
    pass sees the patched view; walrus/codegen still uses the real
    act_info.json, for which set 6 genuinely contains exp/ln/square/copy.
    """
    global _ACT_PATCHED
    if _ACT_PATCHED:
        return
    real = bacc.get_activation_tables

    def only_nle(arch):
        tabs = dict(real(arch))
        return {name: (funcs if name == "natural_log_exp_and_others"
                       else set())
                for name, funcs in tabs.items()}

    bacc.get_activation_tables = only_nle
    _ACT_PATCHED = True


def _pos_encoding_np(n, d):
    pos = np.arange(n, dtype=np.float32)[:, None]
    div = np.exp(np.arange(0, d, 2, dtype=np.float32)
                 * (np.float32(-np.log(np.float32(10000.0))) / np.float32(d)))
    pe = np.zeros((n, d), np.float32)
    pe[:, 0::2] = np.sin(pos * div)
    pe[:, 1::2] = np.cos(pos * div)
    return pe


def build_nc():
    _patch_act_tables()
    nc = bacc.Bacc("TRN2", target_bir_lowering=False, debug=False,
                   num_devices=NCORES)

    Xs = nc.dram_tensor("Xs", [SPC, C, L], F8, kind="ExternalInput")
    W8 = nc.dram_tensor("W8", [C, NCH * 2 * D], F8, kind="ExternalInput")
    PEBTC = nc.dram_tensor("PEBTC", [128, PB_W], BF16, kind="ExternalInput")
    CB = nc.dram_tensor("CB", [128, CB_W], F32, kind="ExternalInput")
    Yout = nc.dram_tensor("Yout", [SPC, PRED], F32, kind="ExternalOutput")

    with tile.TileContext(nc) as tc:
        with (
            tc.tile_pool(name="const", bufs=1) as pc,
            tc.tile_pool(name="hp", bufs=2) as hp,
            tc.tile_pool(name="sqp", bufs=2) as sqp,
            tc.tile_pool(name="vp", bufs=2) as vp,
            tc.tile_pool(name="sm", bufs=2) as sm,
            tc.tile_pool(name="ps", bufs=2, space="PSUM") as ps,
        ):
            # ---- SBUF tiles & DMA order (DMA_ENGINES serializes in this
            # order; arrange so PE conv never starves) ----
            w8 = pc.tile([C, NCH * 2 * D], F8, tag="w8")
            x8 = [pc.tile([C, L], F8, tag=f"x8_{s}", name=f"x8_{s}")
                  for s in range(SPC)]
            pebtc = pc.tile([128, PB_W], BF16, tag="pebtc")
            cb = pc.tile([128, CB_W], F32, tag="cb")

            nc.sync.dma_start(x8[0][:, 0:XSPLIT], Xs.ap()[0][:, 0:XSPLIT])
            nc.sync.dma_start(w8[:], W8.ap())
            nc.sync.dma_start(x8[0][:, XSPLIT:L], Xs.ap()[0][:, XSPLIT:L])
            nc.sync.dma_start(pebtc[:], PEBTC.ap())
            nc.sync.dma_start(x8[1][:], Xs.ap()[1])
            nc.sync.dma_start(x8[2][:], Xs.ap()[2])
            nc.sync.dma_start(x8[3][:], Xs.ap()[3])
            nc.sync.dma_start(cb[:], CB.ap())

            pebt = pebtc[:, 0:NT]
            wexpb = pebtc[0:D + 1, PB_WEXP:PB_WEXP + E * D]
            vwtb = pebtc[:, PB_VWT:PB_VWT + D]
            sqcol = cb[:, CB_SQCOL:CB_SQCOL + 1]
            skcol = cb[:, CB_SKCOL:CB_SKCOL + 1]
            svcol = cb[0:D, CB_SVCOL:CB_SVCOL + 1]
            selab = cb[:, CB_SELAB:CB_SELAB + 2]
            ones128 = cb[:, CB_ONES:CB_ONES + 1]
            onehot = cb[:, CB_ONEHOT:CB_ONEHOT + 1]
            lastm = cb[:, CB_LASTM:CB_LASTM + 1]
            rwt = cb[0:D, CB_RWT:CB_RWT + E]
            qwt = cb[:, CB_QWT:CB_QWT + D]
            kw2 = cb[:, CB_KW:CB_KW + D]
            owt = cb[0:D, CB_OWT:CB_OWT + D]
            mowt = cb[0:D, CB_MOWT:CB_MOWT + D]
            outwt = cb[0:D, CB_OUTWT:CB_OUTWT + PRED]
            id4 = cb[0:SPC, CB_ID4:CB_ID4 + SPC]
            onesr = cb[0:1, CB_ONESR:CB_ONESR + 128]

            ha = pc.tile([D + 1, SPC], F32, tag="ha")
            nc.vector.memset(ha[D:D + 1, :], 1.0)
            epsb = pc.tile([128, 1], F32, tag="epsb")
            nc.vector.memset(epsb[:], EPS)
            ln8b = pc.tile([128, 1], F32, tag="ln8b")
            nc.vector.memset(ln8b[:], math.log(0.125))

            # ---- PE warmup: ramp the p-state while DMAs stream ----
            dum = pc.tile([C, 256], BF16, tag="dum")
            nc.vector.memset(dum[:], 0.0)
            warm = ps.tile([128, 512], F32, tag="vps", name="warm")

            def emit_warm(n):
                for _ in range(n):
                    nc.tensor.matmul(warm[0:C, 0:256], lhsT=dum[:, 0:C],
                                     rhs=dum[:], start=True, stop=True)

            emit_warm(NWARM)

            # ---- conv: all 4 samples back-to-back on PE ----
            w8v = w8[:].rearrange("p (k j d) -> p k j d", k=NCH, j=2)
            h0s, sqs = [], []
            for pair in range(NPAIR):
                h0 = hp.tile([128, NT], F32, tag="h0", name=f"h0_{pair}")
                nc.vector.memset(h0[:, N:NT], 0.0)
                h0s.append(h0)
                sqs.append(sqp.tile([128, NT], F32, tag="sq",
                                    name=f"sq_{pair}"))
            for s in range(SPC):
                pair, s01 = divmod(s, 2)
                o = C * s01
                xv = x8[s][:].rearrange("p (n t) -> p t n", t=S)
                cps = None
                for pi, c0, n0, nn in QCHUNKS:
                    if c0 == 0:
                        cps = ps.tile([C, 512], F32, tag="cps",
                                      name=f"cps{s}{pi}")
                    for k in range(NCH):
                        q, r = divmod(2 * k, S)
                        nc.tensor.matmul(
                            cps[0:C, c0:c0 + nn],
                            lhsT=w8v[:, k],
                            rhs=xv[:, r:r + 2, n0 + q:n0 + q + nn],
                            start=(k == 0), stop=(k == NCH - 1),
                            perf_mode=DR)
                    if c0 != 0:
                        # evict sample psum -> its h0 half (cross-partition
                        # for the pair's second sample), adding pe+bias
                        w = c0 + nn
                        nc.vector.tensor_add(
                            h0s[pair][o:o + C, 512 * pi:512 * pi + w],
                            cps[0:C, 0:w],
                            pebt[o:o + C, 512 * pi:512 * pi + w])
                if s == 0:
                    emit_warm(NBRIDGE)  # bridge the DMA gap, keep pstate
            wsink = sm.tile([C, 1], F32, tag="wsink")
            nc.vector.tensor_copy(wsink[:], warm[0:C, 0:1])

            # ---- attention + tail: both pairs' chains interleaved ----
            def pair_gen(pair):
                h0, sq = h0s[pair], sqs[pair]
                # LN1 per-token stats via PE selector matmuls
                nc.scalar.activation(sq[:], h0[:], AF.Square)
                h0b = vp.tile([128, NT], BF16, tag="h0b")
                nc.vector.tensor_copy(h0b[:], h0[:])
                yield
                stp = ps.tile([128, 128], F32, tag="stp",
                              name=f"stp{pair}")
                for j in range(NJ):
                    nc.tensor.matmul(stp[:, 2 * j:2 * j + 2],
                                     lhsT=h0[:, 128 * j:128 * j + 128],
                                     rhs=selab, start=True, stop=True)
                    nc.tensor.matmul(stp[:, 16 + 2 * j:16 + 2 * j + 2],
                                     lhsT=sq[:, 128 * j:128 * j + 128],
                                     rhs=selab, start=True, stop=True)
                # v projections (bf16): only need h0b; off the stats chain
                vpss = []
                for s01 in range(2):
                    o = C * s01
                    vps = ps.tile([128, 512], F32, tag="vps",
                                  name=f"vps{pair}{s01}")
                    for j in range(NJ):
                        nc.tensor.matmul(
                            vps[:, 64 * j:64 * j + 64],
                            lhsT=h0b[o:o + C, 128 * j:128 * j + 128],
                            rhs=vwtb[o:o + C, :],
                            start=True, stop=True)
                    vpss.append(vps)
                yield
                me = sm.tile([128, 32], F32, tag="me")
                nc.vector.tensor_scalar_mul(me[:], stp[:, 0:32], 1.0 / D)
                mean = me[:, 0:16]
                var = sm.tile([128, 16], F32, tag="var")
                nc.vector.tensor_mul(var[:], mean, mean)
                nc.vector.tensor_sub(var[:], me[:, 16:32], var[:])
                v0s = []
                for s01 in range(2):
                    v0 = vp.tile([128, 512], F32, tag="v0sb",
                                 name=f"v0_{pair}{s01}")
                    nc.scalar.copy(v0[:], vpss[s01][:])
                    v0s.append(v0)
                yield
                # rstd = exp(-0.5*ln(var+eps)); r8 folds the 1/sqrt(D)
                lnv = sm.tile([128, 16], F32, tag="lnv")
                nc.scalar.activation(lnv[:], var[:], AF.Ln, bias=epsb[:])
                rstd = sm.tile([128, 16], F32, tag="rstd")
                nc.scalar.activation(rstd[:], lnv[:], AF.Exp, scale=-0.5)
                r8 = sm.tile([128, 16], F32, tag="r8")
                nc.scalar.activation(r8[:], lnv[:], AF.Exp, scale=-0.5,
                                     bias=ln8b[:])
                yield
                # q for the last token + (mu,r) of the last token
                misc = ps.tile([128, 44], F32, tag="misc",
                               name=f"misc{pair}")
                qe = sm.tile([128, 1], F32, tag="qe")
                for s01 in range(2):
                    o = C * s01
                    nc.tensor.matmul(misc[o:o + C, 0:1], lhsT=qwt[o:o + C, :],
                                     rhs=h0[o:o + C, N - 1:N],
                                     start=True, stop=True)
                nc.tensor.matmul(misc[0:1, 20:22], lhsT=onehot,
                                 rhs=mean[:, 14:16], start=True, stop=True)
                nc.tensor.matmul(misc[0:1, 22:24], lhsT=onehot,
                                 rhs=rstd[:, 14:16], start=True, stop=True)
                yield
                ex4 = sm.tile([1, 4], F32, tag="ex4")
                nc.vector.tensor_copy(ex4[:], misc[0:1, 20:24])
                yield
                for s01 in range(2):
                    o = C * s01
                    nc.tensor.matmul(misc[o:o + C, 24:25],
                                     lhsT=onesr[0:1, 0:C],
                                     rhs=ex4[0:1, s01:s01 + 1],
                                     start=True, stop=True)
                    nc.tensor.matmul(misc[o:o + C, 25:26],
                                     lhsT=onesr[0:1, 0:C],
                                     rhs=ex4[0:1, 2 + s01:3 + s01],
                                     start=True, stop=True)
                yield
                for s01 in range(2):
                    o = C * s01
                    # q_eff = r_last * (q0 - mu_last * Sq)
                    nc.vector.tensor_mul(qe[o:o + C, :], sqcol[o:o + C, :],
                                         misc[o:o + C, 24:25])
                    nc.vector.tensor_sub(qe[o:o + C, :], misc[o:o + C, 0:1],
                                         qe[o:o + C, :])
                    nc.vector.tensor_mul(qe[o:o + C, :], qe[o:o + C, :],
                                         misc[o:o + C, 25:26])
                yield
                for s01 in range(2):
                    o = C * s01
                    nc.tensor.matmul(misc[0:1, 26 + s01:27 + s01],
                                     lhsT=qe[o:o + C, :],
                                     rhs=skcol[o:o + C, :],
                                     start=True, stop=True)
                    nc.tensor.matmul(misc[o:o + C, 1:2],
                                     lhsT=kw2[o:o + C, :],
                                     rhs=qe[o:o + C, :],
                                     start=True, stop=True)
                yield
                qks = sm.tile([128, 1], F32, tag="qks")
                nc.vector.tensor_copy(qks[:], misc[:, 1:2])
                c1r = sm.tile([1, 2], F32, tag="c1r")
                nc.vector.tensor_copy(c1r[:], misc[0:1, 26:28])
                yield
                nc.tensor.matmul(misc[:, 2:4], lhsT=onesr, rhs=c1r[:],
                                 start=True, stop=True)
                for j in range(NJ):
                    for s01 in range(2):
                        o = C * s01
                        nc.tensor.matmul(
                            misc[:, 4 + 2 * j + s01:5 + 2 * j + s01],
                            lhsT=h0[o:o + C, 128 * j:128 * j + 128],
                            rhs=qks[o:o + C, :], start=True, stop=True)
                yield
                mv3 = mean.rearrange("p (j t) -> p j t", t=2)
                tmp = sm.tile([128, 16], F32, tag="tmp")
                nc.vector.tensor_tensor(
                    tmp[:].rearrange("p (j t) -> p j t", t=2), mv3,
                    misc[:, 2:4][:, None].to_broadcast([128, NJ, 2]),
                    op=OP.mult)
                sc = sm.tile([128, 16], F32, tag="sc")
                nc.vector.tensor_sub(sc[:], misc[:, 4:20], tmp[:])
                nc.vector.tensor_mul(sc[:], sc[:], r8[:])
                yield
                exps = sm.tile([128, 16], F32, tag="exps")
                nc.scalar.activation(exps[:], sc[:], AF.Exp)
                nc.vector.tensor_scalar(exps[:, 14:16], exps[:, 14:16],
                                        lastm[:], None, op0=OP.mult)
                yield
                # Z per sample: chunk-reduce then one-matmul partition-reduce
                zs = sm.tile([128, 2], F32, tag="zs")
                nc.vector.tensor_reduce(
                    zs[:], exps[:].rearrange("p (j t) -> p t j", t=2),
                    mybir.AxisListType.X, OP.add)
                yield
                nc.tensor.matmul(misc[0:1, 29:31], lhsT=ones128, rhs=zs[:],
                                 start=True, stop=True)
                yield
                rzr = sm.tile([1, 2], F32, tag="rzr")
                nc.vector.reciprocal(rzr[:], misc[0:1, 29:31])
                yield
                nc.tensor.matmul(misc[:, 32:34], lhsT=onesr, rhs=rzr[:],
                                 start=True, stop=True)
                yield
                # wr = exps/Z * rstd ; g = sum(wr * mu) per sample
                wr = sm.tile([128, 16], F32, tag="wr")
                nc.vector.tensor_tensor(
                    wr[:].rearrange("p (j t) -> p j t", t=2),
                    exps[:].rearrange("p (j t) -> p j t", t=2),
                    misc[:, 32:34][:, None].to_broadcast([128, NJ, 2]),
                    op=OP.mult)
                nc.vector.tensor_mul(wr[:], wr[:], rstd[:])
                gt = sm.tile([128, 16], F32, tag="gt")
                nc.vector.tensor_mul(gt[:], wr[:], mean)
                gs = sm.tile([128, 2], F32, tag="gs")
                nc.vector.tensor_reduce(
                    gs[:], gt[:].rearrange("p (j t) -> p t j", t=2),
                    mybir.AxisListType.X, OP.add)
                yield
                nc.tensor.matmul(misc[0:1, 35:37], lhsT=ones128, rhs=gs[:],
                                 start=True, stop=True)
                yield
                grow = sm.tile([1, 2], F32, tag="grow")
                nc.vector.tensor_copy(grow[:], misc[0:1, 35:37])
                yield
                nc.tensor.matmul(misc[0:C, 37:39], lhsT=onesr[0:1, 0:C],
                                 rhs=grow[:], start=True, stop=True)
                yield
                # attention accumulate + out-proj, per sample
                for s01 in range(2):
                    s = 2 * pair + s01
                    for j in range(NJ):
                        nc.tensor.matmul(
                            misc[0:C, 39 + s01:40 + s01],
                            lhsT=v0s[s01][:, 64 * j:64 * j + 64],
                            rhs=wr[:, 2 * j + s01:2 * j + s01 + 1],
                            start=(j == 0), stop=(j == NJ - 1))
                    yield
                    oc = sm.tile([D, 1], F32, tag="oc")
                    nc.vector.tensor_mul(oc[:], svcol,
                                         misc[0:C, 37 + s01:38 + s01])
                    nc.vector.tensor_sub(oc[:], misc[0:C, 39 + s01:40 + s01],
                                         oc[:])
                    yield
                    nc.tensor.matmul(misc[0:C, 41 + s01:42 + s01],
                                     lhsT=owt, rhs=oc[:],
                                     start=True, stop=True)
                    yield
                    nc.vector.tensor_copy(ha[0:D, s:s + 1],
                                          misc[0:C, 41 + s01:42 + s01])
                    yield
                # ---- tail for this pair's 2 samples ----
                p2 = 2 * pair
                tl = ps.tile([128, 128], F32, tag="stp", name=f"tl{pair}")
                nc.tensor.matmul(tl[0:2, 0:E], lhsT=ha[0:D, p2:p2 + 2],
                                 rhs=rwt, start=True, stop=True)
                hab = sm.tile([D + 1, 2], BF16, tag="hab")
                nc.vector.tensor_copy(hab[:], ha[:, p2:p2 + 2])
                yield
                el = sm.tile([2, E], F32, tag="el")
                nc.scalar.activation(el[:], tl[0:2, 0:E], AF.Exp)
                eop = ps.tile([C, 512], F32, tag="cps", name=f"eop{pair}")
                nc.tensor.matmul(eop[0:2, 0:E * D], lhsT=hab[:], rhs=wexpb,
                                 start=True, stop=True)
                yield
                zr = sm.tile([2, 1], F32, tag="zr")
                nc.vector.tensor_reduce(zr[:], el[:], mybir.AxisListType.X,
                                        OP.add)
                rr = sm.tile([2, 1], F32, tag="rr")
                nc.vector.reciprocal(rr[:], zr[:])
                rw = sm.tile([2, E], F32, tag="rw")
                nc.vector.tensor_scalar(rw[:], el[:], rr[:], None,
                                        op0=OP.mult)
                m8 = sm.tile([2, 8], F32, tag="m8")
                nc.vector.max(m8[:], rw[:])
                msk = sm.tile([2, E], F32, tag="msk")
                nc.vector.tensor_scalar(msk[:], rw[:],
                                        m8[:, TOPK - 1:TOPK], None,
                                        op0=OP.is_ge)
                w4 = sm.tile([2, E], F32, tag="w4")
                nc.vector.tensor_mul(w4[:], rw[:], msk[:])
                yield
                prod = sm.tile([2, E * D], F32, tag="prod")
                nc.vector.tensor_tensor(
                    prod[:].rearrange("p (e d) -> p e d", e=E),
                    eop[0:2, 0:E * D].rearrange("p (e d) -> p e d", e=E),
                    w4[:].to_broadcast([2, E, D]), op=OP.mult)
                moe2 = sm.tile([2, D], F32, tag="moe2")
                nc.vector.tensor_reduce(
                    moe2[:], prod[:].rearrange("p (e d) -> p d e", e=E),
                    mybir.AxisListType.X, OP.add)
                yield
                nc.tensor.transpose(tl[0:D, 8:10], moe2[:], id4[0:2, 0:2])
                yield
                moet = sm.tile([D, 2], F32, tag="moet")
                nc.vector.tensor_copy(moet[:], tl[0:D, 8:10])
                yield
                nc.tensor.matmul(tl[0:D, 16:18], lhsT=mowt, rhs=moet[:],
                                 start=True, stop=True)
                yield
                hm = sm.tile([D, 2], F32, tag="hm")
                nc.vector.tensor_copy(hm[:], tl[0:D, 16:18])
                yield
                nc.tensor.matmul(tl[0:1, 24:26], lhsT=ones128[0:D, :],
                                 rhs=hm[:], start=True, stop=True)
                yield
                mu2 = sm.tile([1, 2], F32, tag="mu2")
                nc.scalar.activation(mu2[:], tl[0:1, 24:26], AF.Copy,
                                     scale=1.0 / D)
                yield
                nc.tensor.matmul(tl[0:D, 28:30], lhsT=onesr[0:1, 0:D],
                                 rhs=mu2[:], start=True, stop=True)
                yield
                hc = sm.tile([D, 2], F32, tag="hc")
                nc.vector.tensor_sub(hc[:], hm[:], tl[0:D, 28:30])
                yield
                sq2 = sm.tile([D, 2], F32, tag="sq2")
                nc.scalar.activation(sq2[:], hc[:], AF.Square)
                yield
                nc.tensor.matmul(tl[0:1, 30:32], lhsT=ones128[0:D, :],
                                 rhs=sq2[:], start=True, stop=True)
                nc.tensor.matmul(tl[0:2, 32:32 + PRED], lhsT=hc[:],
                                 rhs=outwt, start=True, stop=True)
                yield
                lnv2 = sm.tile([1, 2], F32, tag="lnv2")
                nc.scalar.activation(lnv2[:], tl[0:1, 30:32], AF.Ln,
                                     scale=1.0 / D, bias=epsb[0:1, :])
                rstd2 = sm.tile([1, 2], F32, tag="rstd2")
                nc.scalar.activation(rstd2[:], lnv2[:], AF.Exp, scale=-0.5)
                yield
                nc.tensor.matmul(tl[0:2, 10:11], lhsT=rstd2[:],
                                 rhs=onesr[0:1, 0:1], start=True, stop=True)
                yield
                rsc = sm.tile([2, 1], F32, tag="rsc")
                nc.vector.tensor_copy(rsc[:], tl[0:2, 10:11])
                yield
                outp = sm.tile([2, PRED], F32, tag="outp")
                nc.scalar.activation(outp[:], tl[0:2, 32:32 + PRED], AF.Copy,
                                     scale=rsc[:])
                yield
                nc.sync.dma_start(Yout.ap()[p2:p2 + 2], outp[:])

            gens = [pair_gen(0), pair_gen(1)]
            alive = list(gens)
            while alive:
                for g in list(alive):
                    try:
                        next(g)
                    except StopIteration:
                        alive.remove(g)

    nc.compile()
    return nc


_NC_CACHE = {}


def _get_nc():
    if "nc" not in _NC_CACHE:
        _NC_CACHE["nc"] = build_nc()
    return _NC_CACHE["nc"]


def _prep_in_maps(inputs):
    f32 = np.float32
    np_f8 = mybir.dt.np(F8)
    np_bf = mybir.dt.np(BF16)
    X = np.ascontiguousarray(inputs["X"], f32)
    conv_w = np.asarray(inputs["conv_w"], f32)
    conv_b = np.asarray(inputs["conv_b"], f32)
    qw, kw, vw, ow = (np.asarray(inputs[k], f32)
                      for k in ("qw", "kw", "vw", "ow"))
    expert_w = np.asarray(inputs["expert_w"], f32)
    expert_b = np.asarray(inputs["expert_b"], f32)
    router_w = np.asarray(inputs["router_w"], f32)
    moe_out_w = np.asarray(inputs["moe_out_w"], f32)
    out_w = np.asarray(inputs["out_w"], f32)

    # conv weights: [c, (k, j, d)] = conv_w[d, c, 2k+j] * WSC, fp8
    W8 = np.ascontiguousarray(
        conv_w.transpose(1, 2, 0).reshape(C, P * D) * WSC
    ).astype(np_f8)

    # bf16 block: pebt*WSC (doubled rows) + expert weights + vw.T doubled
    pebT = ((_pos_encoding_np(N, D) + conv_b[None, :]) * WSC).T.astype(f32)
    PEBTC = np.zeros((128, PB_W), f32)
    PEBTC[0:D, 0:N] = pebT
    PEBTC[D:128, 0:N] = pebT
    wexp = np.concatenate(
        [expert_w.transpose(2, 0, 1).reshape(D, E * D),
         expert_b.reshape(1, E * D)], axis=0)
    PEBTC[0:D + 1, PB_WEXP:PB_WEXP + E * D] = wexp
    PEBTC[0:D, PB_VWT:PB_VWT + D] = vw.T
    PEBTC[D:128, PB_VWT:PB_VWT + D] = vw.T
    PEBTC = PEBTC.astype(np_bf)

    CBa = np.zeros((128, CB_W), f32)
    dbl = lambda a: np.concatenate([a, a], axis=0)
    CBa[:, CB_SQCOL] = dbl(qw.sum(1))
    CBa[:, CB_SKCOL] = dbl(kw.sum(1))
    CBa[0:D, CB_SVCOL] = vw.sum(1)
    CBa[0:D, CB_SELAB] = 1.0
    CBa[D:128, CB_SELAB + 1] = 1.0
    CBa[:, CB_ONES] = 1.0
    CBa[126, CB_ONEHOT] = 1.0
    CBa[:, CB_LASTM] = 1.0
    CBa[127, CB_LASTM] = 0.0
    CBa[0:D, CB_RWT:CB_RWT + E] = router_w.T
    CBa[:, CB_QWT:CB_QWT + D] = dbl(qw.T)
    CBa[:, CB_KW:CB_KW + D] = dbl(kw)
    CBa[0:D, CB_OWT:CB_OWT + D] = ow.T
    CBa[0:D, CB_MOWT:CB_MOWT + D] = moe_out_w.T
    CBa[0:D, CB_OUTWT:CB_OUTWT + PRED] = out_w.T
    CBa[0:SPC, CB_ID4:CB_ID4 + SPC] = np.eye(SPC, dtype=f32)
    CBa[0, CB_ONESR:CB_ONESR + 128] = 1.0

    common = dict(W8=W8, PEBTC=PEBTC, CB=np.ascontiguousarray(CBa))
    in_maps = []
    for c in range(NCORES):
        m = dict(common)
        m["Xs"] = np.ascontiguousarray(
            X[c * SPC:(c + 1) * SPC]).astype(np_f8)
        in_maps.append(m)
    return in_maps


def kernel(**inputs) -> np.ndarray:
    nc = _get_nc()
    in_maps = _prep_in_maps(inputs)
    res = run_bass_kernel_spmd(nc, in_maps, core_ids=list(range(NCORES)))
    out = np.concatenate([res.results[c]["Yout"] for c in range(NCORES)],
                         axis=0)
    return out.astype(np.float32)
